# revision 9
# baseline (speedup 1.0000x reference)
"""DeepSeek-MoE layer on 8 TRN2 NeuronCores.

Strategy (expert-parallel, host-side dispatch):
  - Router (x @ gate_w.T, softmax, top-2) computed on host — it *is* the
    sharding decision (~0.02% of total FLOPs).
  - Core c computes routed expert c's SwiGLU FFN over the tokens routed to
    it (gathered+padded to a fixed capacity), plus a 512-token chunk of
    shared expert c//4 (each shared expert covers all 2048 tokens, split
    over 4 cores).
  - All matmuls in bf16 (fp32 PSUM accumulation). Combine weights /
    scatter-add applied on host in fp32.

Device kernel layout:
  - Tokens live on the matmul free axis (x stored transposed [H, C]).
  - Weights are streamed as 768KB "4-mi chunks" pre-packed on the host
    into their exact SBUF image, ordered gate/up/down interleaved in
    consumption order — the PE is weight-DMA-paced from ~5us on with no
    phase barriers.
  - Fully fused inner loop per 128-row slice mi of I: 6 gate MMs, 6 up
    MMs, silu (ACT), mul->bf16 (DVE), 6 down MMs accumulating into 6
    PSUM banks (one per 128-row slice of H).
  - Routed capacity is split into two equal column tiles (per-MM cost is
    max(N/2.4GHz, ~64ns floor), so twin tiles beat 512+tail).
  - Output stores go on the ACT HWDGE ring so they never head-of-line
    block weight loads on the sync ring.
"""
import os
import sys
import types

import numpy as np
import ml_dtypes

import concourse.bass as bass
import concourse.tile as tile
import concourse.mybir as mybir
from concourse import bacc
from concourse.bass_utils import run_bass_kernel_spmd

# ---- problem constants (DeepSeekMoE: B=2,S=1024,H=768,I=3072,E=8,NS=2,k=2) --
H = 768          # hidden
I = 3072         # intermediate
E = 8            # routed experts
NS = 2           # shared experts
TOP_K = 2
N_CORES = 8
KH = H // 128    # 6 k-tiles over H
KI = I // 128    # 24 mi-tiles over I
NCH = KI // 4    # 6 weight chunk-groups (4 mi each)
CS = 2048 * NS // N_CORES  # shared-expert tokens per core = 512

BF16 = mybir.dt.bfloat16
F32 = mybir.dt.float32
_bf = ml_dtypes.bfloat16


def _install_ntff_hook():
    """Provide antenv.axon_hooks (missing on this image) so trace=True works."""
    if "antenv.axon_hooks" in sys.modules:
        return
    try:
        from trn_agent_boot.trn_boot import _ntff_profile_via_ctypes
        hook = _ntff_profile_via_ctypes("/opt/axon/libaxon_pjrt.so")
    except Exception:
        hook = None
    mod = types.ModuleType("antenv.axon_hooks")
    mod.get_axon_ntff_profile_hook = lambda: hook
    sys.modules["antenv.axon_hooks"] = mod


def _col_tiles(c):
    if c <= 512:
        return [(0, c)]
    half = (c // 2 + 31) // 32 * 32
    return [(0, half), (half, c - half)]


def _ffn_job(nc, wpool, hpool, sgpool, gupool, ypool, ystage,
             x_sb, wchunks, base, y_dram, n_tiles):
    """One SwiGLU FFN: y = (silu(x Wg) * (x Wu)) Wd for one expert.

    wchunks[base + 3c + {0,1,2}] are the gate/up/down weight chunks for
    mi-group c, pre-packed on host as the exact [128, 3072] SBUF image.
    """
    gu_t = {}
    wd_t = {}
    for c in range(NCH):
        tg = wpool.tile([128, KH, 4, 128], BF16, tag="w")
        nc.sync.dma_start(out=tg, in_=wchunks[base + 3 * c + 0, :, :]
                          .rearrange("p (k j m) -> p k j m", k=KH, j=4))
        tu = wpool.tile([128, KH, 4, 128], BF16, tag="w")
        nc.sync.dma_start(out=tu, in_=wchunks[base + 3 * c + 1, :, :]
                          .rearrange("p (k j m) -> p k j m", k=KH, j=4))
        td = wpool.tile([128, 4, H], BF16, tag="w")
        nc.sync.dma_start(out=td, in_=wchunks[base + 3 * c + 2, :, :]
                          .rearrange("p (j i) -> p j i", j=4))
        gu_t[c] = (tg, tu)
        wd_t[c] = td

    for ti, (n0, nsz) in enumerate(n_tiles):
        ys = [ypool.tile([128, 512], F32, tag="y", name=f"y{hj}")
              for hj in range(KH)]
        for c in range(NCH):
            tg, tu = gu_t[c]
            for j in range(4):
                mi = 4 * c + j
                g = gupool.tile([128, 512], F32, tag="gu")
                u = gupool.tile([128, 512], F32, tag="gu")
                for k in range(KH):
                    nc.tensor.matmul(
                        g[:, :nsz], tg[:, k, j, :], x_sb[:, k, n0:n0 + nsz],
                        start=(k == 0), stop=(k == KH - 1))
                for k in range(KH):
                    nc.tensor.matmul(
                        u[:, :nsz], tu[:, k, j, :], x_sb[:, k, n0:n0 + nsz],
                        start=(k == 0), stop=(k == KH - 1))
                sg = sgpool.tile([128, 512], F32, tag="sg")
                nc.scalar.activation(sg[:, :nsz], g[:, :nsz],
                                     mybir.ActivationFunctionType.Silu)
                h = hpool.tile([128, 512], BF16, tag="h")
                nc.vector.tensor_mul(h[:, :nsz], sg[:, :nsz], u[:, :nsz])
                for hj in range(KH):
                    nc.tensor.matmul(
                        ys[hj][:, :nsz], wd_t[c][:, j, hj * 128:(hj + 1) * 128],
                        h[:, :nsz],
                        start=(mi == 0), stop=(mi == KI - 1))
        for hj in range(KH):
            yst = ystage.tile([128, 512], F32, tag="yst")
            nc.scalar.copy(yst[:, :nsz], ys[hj][:, :nsz])
            nc.scalar.dma_start(
                out=y_dram[hj * 128:(hj + 1) * 128, n0:n0 + nsz],
                in_=yst[:, :nsz])


def build_nc(cr):
    """Build the SPMD program. cr = routed-token capacity (multiple of 32)."""
    nc = bacc.Bacc(None, target_bir_lowering=False)
    xr = nc.dram_tensor("xr", [H, cr], BF16, kind="ExternalInput")
    xs = nc.dram_tensor("xs", [H, CS], BF16, kind="ExternalInput")
    wch = nc.dram_tensor("wch", [6 * NCH, 128, KH * 512], BF16,
                         kind="ExternalInput")
    yr = nc.dram_tensor("yr", [H, cr], F32, kind="ExternalOutput")
    ys = nc.dram_tensor("ys", [H, CS], F32, kind="ExternalOutput")

    with tile.TileContext(nc) as tc:
        with tc.tile_pool(name="wpool", bufs=24) as wpool, \
             tc.tile_pool(name="xpool", bufs=1) as xpool, \
             tc.tile_pool(name="hpool", bufs=8) as hpool, \
             tc.tile_pool(name="sgpool", bufs=4) as sgpool, \
             tc.tile_pool(name="ystage", bufs=4) as ystage, \
             tc.tile_pool(name="gupool", bufs=2, space="PSUM") as gupool, \
             tc.tile_pool(name="ypool", bufs=6, space="PSUM") as ypool:
            xr_sb = xpool.tile([128, KH, cr], BF16, tag="xr")
            xs_sb = xpool.tile([128, KH, CS], BF16, tag="xs")
            for k in range(KH):
                nc.sync.dma_start(
                    out=xr_sb[:, k, :], in_=xr[k * 128:(k + 1) * 128, :])
            for k in range(KH):
                nc.sync.dma_start(
                    out=xs_sb[:, k, :], in_=xs[k * 128:(k + 1) * 128, :])

            _ffn_job(nc, wpool, hpool, sgpool, gupool, ypool, ystage,
                     xr_sb, wch, 0, yr, _col_tiles(cr))
            _ffn_job(nc, wpool, hpool, sgpool, gupool, ypool, ystage,
                     xs_sb, wch, 3 * NCH, ys, _col_tiles(CS))
    nc.finalize()
    return nc


def _chunk_gu(wT):
    """[H, I] lhsT-layout weight -> [NCH, 128, 3072] SBUF chunk images.
    chunk[c][p, k*512 + j*128 + m] = wT[k*128 + p, (4c+j)*128 + m]"""
    a = wT.reshape(KH, 128, NCH, 4 * 128)        # [k, p, c, jm]
    return np.ascontiguousarray(a.transpose(2, 1, 0, 3)).reshape(NCH, 128, KH * 512)


def _chunk_wd(wdT):
    """[I, H] lhsT-layout down weight -> [NCH, 128, 3072] chunk images.
    chunk[c][p, j*768 + i] = wdT[(4c+j)*128 + p, i]"""
    a = wdT.reshape(NCH, 4, 128, H)              # [c, j, p, i]
    return np.ascontiguousarray(a.transpose(0, 2, 1, 3)).reshape(NCH, 128, 4 * H)


def _pack_chunks(gT, uT, dT):
    """Interleave gate/up/down chunks in consumption order -> [18, 128, 3072]."""
    g = _chunk_gu(gT)
    u = _chunk_gu(uT)
    d = _chunk_wd(dT)
    out = np.empty((3 * NCH, 128, KH * 512), _bf)
    out[0::3] = g
    out[1::3] = u
    out[2::3] = d
    return out


_NC_CACHE = {}


def kernel(hidden_states, gate_w, shared_gate, shared_up, shared_down,
           routed_gate, routed_up, routed_down):
    B, S, _ = hidden_states.shape
    T = B * S
    x = np.asarray(hidden_states, np.float32).reshape(T, H)

    # ---- host router (mirrors reference math; fp64 softmax for stability) --
    logits = x @ np.asarray(gate_w, np.float32).T                    # [T, E]
    lg = logits.astype(np.float64)
    sc = np.exp(lg - lg.max(1, keepdims=True))
    sc /= sc.sum(1, keepdims=True)
    topk_idx = np.argsort(-sc, axis=1, kind="stable")[:, :TOP_K]     # [T, k]
    topk_w = np.take_along_axis(sc, topk_idx, axis=1)
    topk_w = topk_w / (topk_w.sum(1, keepdims=True) + 1e-8)          # [T, k]

    tok_lists = []
    tok_weights = []
    for e in range(E):
        sel = (topk_idx == e)
        toks = np.where(sel.any(1))[0]
        w = (topk_w * sel)[toks].sum(1).astype(np.float32)
        tok_lists.append(toks)
        tok_weights.append(w)
    max_n = max(len(t) for t in tok_lists)
    cr = max(64, -(-max_n // 32) * 32)

    # ---- per-core inputs -------------------------------------------------
    x_bf = x.astype(_bf)
    shared_packs = []
    for s in range(NS):
        sgT = np.ascontiguousarray(np.asarray(shared_gate[s], np.float32).T).astype(_bf)
        suT = np.ascontiguousarray(np.asarray(shared_up[s], np.float32).T).astype(_bf)
        sdT = np.ascontiguousarray(np.asarray(shared_down[s], np.float32).T).astype(_bf)
        shared_packs.append(_pack_chunks(sgT, suT, sdT))

    in_maps = []
    for c in range(N_CORES):
        toks = tok_lists[c]
        xr = np.zeros((H, cr), _bf)
        xr[:, :len(toks)] = x_bf[toks].T
        s = c // (N_CORES // NS)
        q = c % (N_CORES // NS)
        xs_ = np.ascontiguousarray(x_bf[q * CS:(q + 1) * CS].T)
        rgT = np.ascontiguousarray(np.asarray(routed_gate[c], np.float32).T).astype(_bf)
        ruT = np.ascontiguousarray(np.asarray(routed_up[c], np.float32).T).astype(_bf)
        rdT = np.ascontiguousarray(np.asarray(routed_down[c], np.float32).T).astype(_bf)
        wch = np.concatenate([_pack_chunks(rgT, ruT, rdT), shared_packs[s]])
        in_maps.append({"xr": xr, "xs": xs_, "wch": wch})

    # ---- build + run on 8 cores -----------------------------------------
    if cr not in _NC_CACHE:
        _NC_CACHE[cr] = build_nc(cr)
    nc = _NC_CACHE[cr]

    trace = bool(int(os.environ.get("MOE_TRACE", "0")))
    kw = {}
    if trace:
        _install_ntff_hook()
        kw = dict(trace=True, trace_cores=list(range(N_CORES)))
    res = run_bass_kernel_spmd(nc, in_maps, core_ids=list(range(N_CORES)), **kw)
    if trace:
        print(f"HW exec time: {res.exec_time_ns} ns")

    # ---- host combine ----------------------------------------------------
    out = np.zeros((T, H), np.float32)
    for c in range(N_CORES):
        toks = tok_lists[c]
        yrT = res.results[c]["yr"]                                   # [H, cr]
        out[toks] += yrT[:, :len(toks)].T * tok_weights[c][:, None]
        q = c % (N_CORES // NS)
        out[q * CS:(q + 1) * CS] += res.results[c]["ys"].T / NS
    return out.reshape(B, S, H)


# revision 13
# speedup vs baseline: 1.0190x; 1.0190x over previous
"""DeepSeek-MoE layer on 8 TRN2 NeuronCores.

Strategy (expert-parallel, host-side dispatch):
  - Router (x @ gate_w.T, softmax, top-2) computed on host — it *is* the
    sharding decision (~0.02% of total FLOPs).
  - Core c computes routed expert c's SwiGLU FFN over the tokens routed to
    it (gathered+padded to a fixed capacity), plus a 512-token chunk of
    shared expert c//4 (each shared expert covers all 2048 tokens, split
    over 4 cores).
  - All matmuls in bf16 (fp32 PSUM accumulation). Combine weights /
    scatter-add applied on host in fp32.

Device kernel layout:
  - Tokens live on the matmul free axis (x stored transposed [H, C]).
  - Weights are streamed as 768KB "4-mi chunks" pre-packed on the host
    into their exact SBUF image, ordered gate/up/down interleaved in
    consumption order — the PE is weight-DMA-paced from ~5us on with no
    phase barriers.
  - Fully fused inner loop per 128-row slice mi of I: 6 gate MMs, 6 up
    MMs, silu (ACT), mul->bf16 (DVE), 6 down MMs accumulating into 6
    PSUM banks (one per 128-row slice of H).
  - Routed capacity is split into two equal column tiles (per-MM cost is
    max(N/2.4GHz, ~64ns floor), so twin tiles beat 512+tail).
  - Output stores go on the ACT HWDGE ring so they never head-of-line
    block weight loads on the sync ring.
"""
import os
import sys
import types

import numpy as np
import ml_dtypes

import concourse.bass as bass
import concourse.tile as tile
import concourse.mybir as mybir
from concourse import bacc
from concourse.bass_utils import run_bass_kernel_spmd

# ---- problem constants (DeepSeekMoE: B=2,S=1024,H=768,I=3072,E=8,NS=2,k=2) --
H = 768          # hidden
I = 3072         # intermediate
E = 8            # routed experts
NS = 2           # shared experts
TOP_K = 2
N_CORES = 8
KH = H // 128    # 6 k-tiles over H
KI = I // 128    # 24 mi-tiles over I
NCH = KI // 4    # 6 weight chunk-groups (4 mi each)
CS = 2048 * NS // N_CORES  # shared-expert tokens per core = 512

BF16 = mybir.dt.bfloat16
F32 = mybir.dt.float32
_bf = ml_dtypes.bfloat16


def _install_ntff_hook():
    """Provide antenv.axon_hooks (missing on this image) so trace=True works."""
    if "antenv.axon_hooks" in sys.modules:
        return
    try:
        from trn_agent_boot.trn_boot import _ntff_profile_via_ctypes
        hook = _ntff_profile_via_ctypes("/opt/axon/libaxon_pjrt.so")
    except Exception:
        hook = None
    mod = types.ModuleType("antenv.axon_hooks")
    mod.get_axon_ntff_profile_hook = lambda: hook
    sys.modules["antenv.axon_hooks"] = mod


def _col_tiles(c):
    if c <= 512:
        return [(0, c)]
    half = (c // 2 + 31) // 32 * 32
    return [(0, half), (half, c - half)]


def _ffn_job(nc, wpool, hpool, sgpool, gupool, ypool, ystage,
             x_sb, wchunks, base, y_dram, n_tiles):
    """One SwiGLU FFN: y = (silu(x Wg) * (x Wu)) Wd for one expert.

    wchunks[base + 3c + {0,1,2}] are the gate/up/down weight chunks for
    mi-group c, pre-packed on host as the exact [128, 3072] SBUF image.
    """
    gu_t = {}
    wd_t = {}
    for c in range(NCH):
        tg = wpool.tile([128, KH, 4, 128], BF16, tag="w")
        nc.sync.dma_start(out=tg, in_=wchunks[base + 3 * c + 0, :, :]
                          .rearrange("p (k j m) -> p k j m", k=KH, j=4))
        tu = wpool.tile([128, KH, 4, 128], BF16, tag="w")
        nc.sync.dma_start(out=tu, in_=wchunks[base + 3 * c + 1, :, :]
                          .rearrange("p (k j m) -> p k j m", k=KH, j=4))
        td = wpool.tile([128, 4, H], BF16, tag="w")
        nc.sync.dma_start(out=td, in_=wchunks[base + 3 * c + 2, :, :]
                          .rearrange("p (j i) -> p j i", j=4))
        gu_t[c] = (tg, tu)
        wd_t[c] = td

    for ti, (n0, nsz) in enumerate(n_tiles):
        ys = [ypool.tile([128, 512], F32, tag="y", name=f"y{hj}")
              for hj in range(KH)]
        for c in range(NCH):
            tg, tu = gu_t[c]
            hs = {}
            # all gate/up of the 4-mi group first, then the 4 downs — the
            # PE is in-order, so this gives the group's down-chunk DMA a
            # ~5us window instead of stalling the stream
            for j in range(4):
                g = gupool.tile([128, 512], F32, tag="gu")
                u = gupool.tile([128, 512], F32, tag="gu")
                for k in range(KH):
                    nc.tensor.matmul(
                        g[:, :nsz], tg[:, k, j, :], x_sb[:, k, n0:n0 + nsz],
                        start=(k == 0), stop=(k == KH - 1))
                for k in range(KH):
                    nc.tensor.matmul(
                        u[:, :nsz], tu[:, k, j, :], x_sb[:, k, n0:n0 + nsz],
                        start=(k == 0), stop=(k == KH - 1))
                sg = sgpool.tile([128, 512], F32, tag="sg")
                nc.scalar.activation(sg[:, :nsz], g[:, :nsz],
                                     mybir.ActivationFunctionType.Silu)
                h = hpool.tile([128, 512], BF16, tag="h")
                nc.vector.tensor_mul(h[:, :nsz], sg[:, :nsz], u[:, :nsz])
                hs[j] = h
            for j in range(4):
                mi = 4 * c + j
                for hj in range(KH):
                    nc.tensor.matmul(
                        ys[hj][:, :nsz], wd_t[c][:, j, hj * 128:(hj + 1) * 128],
                        hs[j][:, :nsz],
                        start=(mi == 0), stop=(mi == KI - 1))
        for hj in range(KH):
            yst = ystage.tile([128, 512], BF16, tag="yst")
            nc.scalar.copy(yst[:, :nsz], ys[hj][:, :nsz])
            nc.scalar.dma_start(
                out=y_dram[hj * 128:(hj + 1) * 128, n0:n0 + nsz],
                in_=yst[:, :nsz])


def build_nc(cr):
    """Build the SPMD program. cr = routed-token capacity (multiple of 32)."""
    nc = bacc.Bacc(None, target_bir_lowering=False)
    xr = nc.dram_tensor("xr", [H, cr], BF16, kind="ExternalInput")
    xs = nc.dram_tensor("xs", [H, CS], BF16, kind="ExternalInput")
    wch = nc.dram_tensor("wch", [6 * NCH, 128, KH * 512], BF16,
                         kind="ExternalInput")
    yr = nc.dram_tensor("yr", [H, cr], BF16, kind="ExternalOutput")
    ys = nc.dram_tensor("ys", [H, CS], BF16, kind="ExternalOutput")

    with tile.TileContext(nc) as tc:
        with tc.tile_pool(name="wpool", bufs=24) as wpool, \
             tc.tile_pool(name="xpool", bufs=1) as xpool, \
             tc.tile_pool(name="hpool", bufs=8) as hpool, \
             tc.tile_pool(name="sgpool", bufs=4) as sgpool, \
             tc.tile_pool(name="ystage", bufs=4) as ystage, \
             tc.tile_pool(name="gupool", bufs=2, space="PSUM") as gupool, \
             tc.tile_pool(name="ypool", bufs=6, space="PSUM") as ypool:
            # x loads ride the ACT HWDGE ring so they stream in parallel
            # with the weight chunks on the sync ring at kernel start
            xr_sb = xpool.tile([128, KH, cr], BF16, tag="xr")
            xs_sb = xpool.tile([128, KH, CS], BF16, tag="xs")
            for k in range(KH):
                nc.scalar.dma_start(
                    out=xr_sb[:, k, :], in_=xr[k * 128:(k + 1) * 128, :])
            for k in range(KH):
                nc.scalar.dma_start(
                    out=xs_sb[:, k, :], in_=xs[k * 128:(k + 1) * 128, :])

            _ffn_job(nc, wpool, hpool, sgpool, gupool, ypool, ystage,
                     xr_sb, wch, 0, yr, _col_tiles(cr))
            _ffn_job(nc, wpool, hpool, sgpool, gupool, ypool, ystage,
                     xs_sb, wch, 3 * NCH, ys, _col_tiles(CS))
    nc.finalize()
    return nc


def _chunk_gu(wT):
    """[H, I] lhsT-layout weight -> [NCH, 128, 3072] SBUF chunk images.
    chunk[c][p, k*512 + j*128 + m] = wT[k*128 + p, (4c+j)*128 + m]"""
    a = wT.reshape(KH, 128, NCH, 4 * 128)        # [k, p, c, jm]
    return np.ascontiguousarray(a.transpose(2, 1, 0, 3)).reshape(NCH, 128, KH * 512)


def _chunk_wd(wdT):
    """[I, H] lhsT-layout down weight -> [NCH, 128, 3072] chunk images.
    chunk[c][p, j*768 + i] = wdT[(4c+j)*128 + p, i]"""
    a = wdT.reshape(NCH, 4, 128, H)              # [c, j, p, i]
    return np.ascontiguousarray(a.transpose(0, 2, 1, 3)).reshape(NCH, 128, 4 * H)


def _pack_chunks(gT, uT, dT):
    """Interleave gate/up/down chunks in consumption order -> [18, 128, 3072]."""
    g = _chunk_gu(gT)
    u = _chunk_gu(uT)
    d = _chunk_wd(dT)
    out = np.empty((3 * NCH, 128, KH * 512), _bf)
    out[0::3] = g
    out[1::3] = u
    out[2::3] = d
    return out


_NC_CACHE = {}


def kernel(hidden_states, gate_w, shared_gate, shared_up, shared_down,
           routed_gate, routed_up, routed_down):
    B, S, _ = hidden_states.shape
    T = B * S
    x = np.asarray(hidden_states, np.float32).reshape(T, H)

    # ---- host router (mirrors reference math; fp64 softmax for stability) --
    logits = x @ np.asarray(gate_w, np.float32).T                    # [T, E]
    lg = logits.astype(np.float64)
    sc = np.exp(lg - lg.max(1, keepdims=True))
    sc /= sc.sum(1, keepdims=True)
    topk_idx = np.argsort(-sc, axis=1, kind="stable")[:, :TOP_K]     # [T, k]
    topk_w = np.take_along_axis(sc, topk_idx, axis=1)
    topk_w = topk_w / (topk_w.sum(1, keepdims=True) + 1e-8)          # [T, k]

    tok_lists = []
    tok_weights = []
    for e in range(E):
        sel = (topk_idx == e)
        toks = np.where(sel.any(1))[0]
        w = (topk_w * sel)[toks].sum(1).astype(np.float32)
        tok_lists.append(toks)
        tok_weights.append(w)
    max_n = max(len(t) for t in tok_lists)
    cr = max(64, -(-max_n // 32) * 32)

    # ---- per-core inputs -------------------------------------------------
    x_bf = x.astype(_bf)
    shared_packs = []
    for s in range(NS):
        sgT = np.ascontiguousarray(np.asarray(shared_gate[s], np.float32).T).astype(_bf)
        suT = np.ascontiguousarray(np.asarray(shared_up[s], np.float32).T).astype(_bf)
        sdT = np.ascontiguousarray(np.asarray(shared_down[s], np.float32).T).astype(_bf)
        shared_packs.append(_pack_chunks(sgT, suT, sdT))

    in_maps = []
    for c in range(N_CORES):
        toks = tok_lists[c]
        xr = np.zeros((H, cr), _bf)
        xr[:, :len(toks)] = x_bf[toks].T
        s = c // (N_CORES // NS)
        q = c % (N_CORES // NS)
        xs_ = np.ascontiguousarray(x_bf[q * CS:(q + 1) * CS].T)
        rgT = np.ascontiguousarray(np.asarray(routed_gate[c], np.float32).T).astype(_bf)
        ruT = np.ascontiguousarray(np.asarray(routed_up[c], np.float32).T).astype(_bf)
        rdT = np.ascontiguousarray(np.asarray(routed_down[c], np.float32).T).astype(_bf)
        wch = np.concatenate([_pack_chunks(rgT, ruT, rdT), shared_packs[s]])
        in_maps.append({"xr": xr, "xs": xs_, "wch": wch})

    # ---- build + run on 8 cores -----------------------------------------
    if cr not in _NC_CACHE:
        _NC_CACHE[cr] = build_nc(cr)
    nc = _NC_CACHE[cr]

    trace = bool(int(os.environ.get("MOE_TRACE", "0")))
    kw = {}
    if trace:
        _install_ntff_hook()
        kw = dict(trace=True, trace_cores=list(range(N_CORES)))
    res = run_bass_kernel_spmd(nc, in_maps, core_ids=list(range(N_CORES)), **kw)
    if trace:
        print(f"HW exec time: {res.exec_time_ns} ns")

    # ---- host combine ----------------------------------------------------
    out = np.zeros((T, H), np.float32)
    for c in range(N_CORES):
        toks = tok_lists[c]
        yrT = res.results[c]["yr"].astype(np.float32)                # [H, cr]
        out[toks] += yrT[:, :len(toks)].T * tok_weights[c][:, None]
        q = c % (N_CORES // NS)
        out[q * CS:(q + 1) * CS] += res.results[c]["ys"].astype(np.float32).T / NS
    return out.reshape(B, S, H)


# revision 14
# speedup vs baseline: 1.0725x; 1.0525x over previous
"""DeepSeek-MoE layer on 8 TRN2 NeuronCores.

Strategy (expert-parallel, host-side dispatch):
  - Router (x @ gate_w.T, softmax, top-2) computed on host — it *is* the
    sharding decision (~0.02% of total FLOPs).
  - Core c computes routed expert c's SwiGLU FFN over the tokens routed to
    it (gathered+padded to a fixed capacity), plus a 512-token chunk of
    shared expert c//4 (each shared expert covers all 2048 tokens, split
    over 4 cores).
  - All matmuls in bf16 (fp32 PSUM accumulation). Combine weights /
    scatter-add applied on host in fp32.

Device kernel layout:
  - Tokens live on the matmul free axis (x stored transposed [H, C]).
  - Weights are streamed as 768KB "4-mi chunks" pre-packed on the host
    into their exact SBUF image, ordered gate/up/down interleaved in
    consumption order — the PE is weight-DMA-paced from ~5us on with no
    phase barriers.
  - Fully fused inner loop per 128-row slice mi of I: 6 gate MMs, 6 up
    MMs, silu (ACT), mul->bf16 (DVE), 6 down MMs accumulating into 6
    PSUM banks (one per 128-row slice of H).
  - Routed capacity is split into two equal column tiles (per-MM cost is
    max(N/2.4GHz, ~64ns floor), so twin tiles beat 512+tail).
  - Output stores go on the ACT HWDGE ring so they never head-of-line
    block weight loads on the sync ring.
"""
import os
import sys
import types

import numpy as np
import ml_dtypes

import concourse.bass as bass
import concourse.tile as tile
import concourse.mybir as mybir
from concourse import bacc
from concourse.bass_utils import run_bass_kernel_spmd

# ---- problem constants (DeepSeekMoE: B=2,S=1024,H=768,I=3072,E=8,NS=2,k=2) --
H = 768          # hidden
I = 3072         # intermediate
E = 8            # routed experts
NS = 2           # shared experts
TOP_K = 2
N_CORES = 8
KH = H // 128    # 6 k-tiles over H
KI = I // 128    # 24 mi-tiles over I
NCH = KI // 4    # 6 weight chunk-groups (4 mi each)
CS = 2048 * NS // N_CORES  # shared-expert tokens per core = 512

BF16 = mybir.dt.bfloat16
F32 = mybir.dt.float32
_bf = ml_dtypes.bfloat16


def _install_ntff_hook():
    """Provide antenv.axon_hooks (missing on this image) so trace=True works."""
    if "antenv.axon_hooks" in sys.modules:
        return
    try:
        from trn_agent_boot.trn_boot import _ntff_profile_via_ctypes
        hook = _ntff_profile_via_ctypes("/opt/axon/libaxon_pjrt.so")
    except Exception:
        hook = None
    mod = types.ModuleType("antenv.axon_hooks")
    mod.get_axon_ntff_profile_hook = lambda: hook
    sys.modules["antenv.axon_hooks"] = mod


def _col_tiles(c):
    if c <= 512:
        return [(0, c)]
    half = (c // 2 + 31) // 32 * 32
    return [(0, half), (half, c - half)]


def _ffn_job(nc, wpool, hpool, sgpool, gupool, ypool, ystage,
             x_sb, wchunks, base, y_dram, n_tiles):
    """One SwiGLU FFN: y = (silu(x Wg) * (x Wu)) Wd for one expert.

    wchunks[base + 3c + {0,1,2}] are the gate/up/down weight chunks for
    mi-group c, pre-packed on host as the exact [128, 3072] SBUF image.
    """
    gu_t = {}
    wd_t = {}
    for c in range(NCH):
        tg = wpool.tile([128, KH, 4, 128], BF16, tag="w")
        nc.sync.dma_start(out=tg, in_=wchunks[base + 3 * c + 0, :, :]
                          .rearrange("p (k j m) -> p k j m", k=KH, j=4))
        tu = wpool.tile([128, KH, 4, 128], BF16, tag="w")
        nc.sync.dma_start(out=tu, in_=wchunks[base + 3 * c + 1, :, :]
                          .rearrange("p (k j m) -> p k j m", k=KH, j=4))
        td = wpool.tile([128, 4, H], BF16, tag="w")
        nc.sync.dma_start(out=td, in_=wchunks[base + 3 * c + 2, :, :]
                          .rearrange("p (j i) -> p j i", j=4))
        gu_t[c] = (tg, tu)
        wd_t[c] = td

    for ti, (n0, nsz) in enumerate(n_tiles):
        ys = [ypool.tile([128, 512], F32, tag="y", name=f"y{hj}")
              for hj in range(KH)]
        for c in range(NCH):
            tg, tu = gu_t[c]
            hs = {}
            # all gate/up of the 4-mi group first, then the 4 downs — the
            # PE is in-order, so this gives the group's down-chunk DMA a
            # ~5us window instead of stalling the stream
            for j in range(4):
                g = gupool.tile([128, 512], F32, tag="gu")
                u = gupool.tile([128, 512], F32, tag="gu")
                for k in range(KH):
                    nc.tensor.matmul(
                        g[:, :nsz], tg[:, k, j, :], x_sb[:, k, n0:n0 + nsz],
                        start=(k == 0), stop=(k == KH - 1))
                for k in range(KH):
                    nc.tensor.matmul(
                        u[:, :nsz], tu[:, k, j, :], x_sb[:, k, n0:n0 + nsz],
                        start=(k == 0), stop=(k == KH - 1))
                sg = sgpool.tile([128, 512], F32, tag="sg")
                nc.scalar.activation(sg[:, :nsz], g[:, :nsz],
                                     mybir.ActivationFunctionType.Silu)
                h = hpool.tile([128, 512], BF16, tag="h")
                nc.vector.tensor_mul(h[:, :nsz], sg[:, :nsz], u[:, :nsz])
                hs[j] = h
            for j in range(4):
                mi = 4 * c + j
                for hj in range(KH):
                    nc.tensor.matmul(
                        ys[hj][:, :nsz], wd_t[c][:, j, hj * 128:(hj + 1) * 128],
                        hs[j][:, :nsz],
                        start=(mi == 0), stop=(mi == KI - 1))
        # split the drain-out across engines/rings: copies alternate
        # ACT/DVE, stores alternate ACT/sync HWDGE rings — otherwise the
        # six copy+store pairs serialize on one engine and stall both the
        # next tile's silu chain and the kernel tail
        for hj in range(KH):
            yst = ystage.tile([128, 512], BF16, tag="yst")
            if hj % 2 == 0:
                nc.scalar.copy(yst[:, :nsz], ys[hj][:, :nsz])
            else:
                nc.vector.tensor_copy(yst[:, :nsz], ys[hj][:, :nsz])
            eng = nc.scalar if hj % 2 == 0 else nc.sync
            eng.dma_start(
                out=y_dram[hj * 128:(hj + 1) * 128, n0:n0 + nsz],
                in_=yst[:, :nsz])


def build_nc(cr):
    """Build the SPMD program. cr = routed-token capacity (multiple of 32)."""
    nc = bacc.Bacc(None, target_bir_lowering=False)
    xr = nc.dram_tensor("xr", [H, cr], BF16, kind="ExternalInput")
    xs = nc.dram_tensor("xs", [H, CS], BF16, kind="ExternalInput")
    wch = nc.dram_tensor("wch", [6 * NCH, 128, KH * 512], BF16,
                         kind="ExternalInput")
    yr = nc.dram_tensor("yr", [H, cr], BF16, kind="ExternalOutput")
    ys = nc.dram_tensor("ys", [H, CS], BF16, kind="ExternalOutput")

    with tile.TileContext(nc) as tc:
        with tc.tile_pool(name="wpool", bufs=24) as wpool, \
             tc.tile_pool(name="xpool", bufs=1) as xpool, \
             tc.tile_pool(name="hpool", bufs=8) as hpool, \
             tc.tile_pool(name="sgpool", bufs=4) as sgpool, \
             tc.tile_pool(name="ystage", bufs=4) as ystage, \
             tc.tile_pool(name="gupool", bufs=2, space="PSUM") as gupool, \
             tc.tile_pool(name="ypool", bufs=6, space="PSUM") as ypool:
            # x loads ride the ACT HWDGE ring so they stream in parallel
            # with the weight chunks on the sync ring at kernel start
            xr_sb = xpool.tile([128, KH, cr], BF16, tag="xr")
            xs_sb = xpool.tile([128, KH, CS], BF16, tag="xs")
            for k in range(KH):
                nc.scalar.dma_start(
                    out=xr_sb[:, k, :], in_=xr[k * 128:(k + 1) * 128, :])
            for k in range(KH):
                nc.scalar.dma_start(
                    out=xs_sb[:, k, :], in_=xs[k * 128:(k + 1) * 128, :])

            _ffn_job(nc, wpool, hpool, sgpool, gupool, ypool, ystage,
                     xr_sb, wch, 0, yr, _col_tiles(cr))
            _ffn_job(nc, wpool, hpool, sgpool, gupool, ypool, ystage,
                     xs_sb, wch, 3 * NCH, ys, _col_tiles(CS))
    nc.finalize()
    return nc


def _chunk_gu(wT):
    """[H, I] lhsT-layout weight -> [NCH, 128, 3072] SBUF chunk images.
    chunk[c][p, k*512 + j*128 + m] = wT[k*128 + p, (4c+j)*128 + m]"""
    a = wT.reshape(KH, 128, NCH, 4 * 128)        # [k, p, c, jm]
    return np.ascontiguousarray(a.transpose(2, 1, 0, 3)).reshape(NCH, 128, KH * 512)


def _chunk_wd(wdT):
    """[I, H] lhsT-layout down weight -> [NCH, 128, 3072] chunk images.
    chunk[c][p, j*768 + i] = wdT[(4c+j)*128 + p, i]"""
    a = wdT.reshape(NCH, 4, 128, H)              # [c, j, p, i]
    return np.ascontiguousarray(a.transpose(0, 2, 1, 3)).reshape(NCH, 128, 4 * H)


def _pack_chunks(gT, uT, dT):
    """Interleave gate/up/down chunks in consumption order -> [18, 128, 3072]."""
    g = _chunk_gu(gT)
    u = _chunk_gu(uT)
    d = _chunk_wd(dT)
    out = np.empty((3 * NCH, 128, KH * 512), _bf)
    out[0::3] = g
    out[1::3] = u
    out[2::3] = d
    return out


_NC_CACHE = {}


def kernel(hidden_states, gate_w, shared_gate, shared_up, shared_down,
           routed_gate, routed_up, routed_down):
    B, S, _ = hidden_states.shape
    T = B * S
    x = np.asarray(hidden_states, np.float32).reshape(T, H)

    # ---- host router (mirrors reference math; fp64 softmax for stability) --
    logits = x @ np.asarray(gate_w, np.float32).T                    # [T, E]
    lg = logits.astype(np.float64)
    sc = np.exp(lg - lg.max(1, keepdims=True))
    sc /= sc.sum(1, keepdims=True)
    topk_idx = np.argsort(-sc, axis=1, kind="stable")[:, :TOP_K]     # [T, k]
    topk_w = np.take_along_axis(sc, topk_idx, axis=1)
    topk_w = topk_w / (topk_w.sum(1, keepdims=True) + 1e-8)          # [T, k]

    tok_lists = []
    tok_weights = []
    for e in range(E):
        sel = (topk_idx == e)
        toks = np.where(sel.any(1))[0]
        w = (topk_w * sel)[toks].sum(1).astype(np.float32)
        tok_lists.append(toks)
        tok_weights.append(w)
    max_n = max(len(t) for t in tok_lists)
    cr = max(64, -(-max_n // 32) * 32)

    # ---- per-core inputs -------------------------------------------------
    x_bf = x.astype(_bf)
    shared_packs = []
    for s in range(NS):
        sgT = np.ascontiguousarray(np.asarray(shared_gate[s], np.float32).T).astype(_bf)
        suT = np.ascontiguousarray(np.asarray(shared_up[s], np.float32).T).astype(_bf)
        sdT = np.ascontiguousarray(np.asarray(shared_down[s], np.float32).T).astype(_bf)
        shared_packs.append(_pack_chunks(sgT, suT, sdT))

    in_maps = []
    for c in range(N_CORES):
        toks = tok_lists[c]
        xr = np.zeros((H, cr), _bf)
        xr[:, :len(toks)] = x_bf[toks].T
        s = c // (N_CORES // NS)
        q = c % (N_CORES // NS)
        xs_ = np.ascontiguousarray(x_bf[q * CS:(q + 1) * CS].T)
        rgT = np.ascontiguousarray(np.asarray(routed_gate[c], np.float32).T).astype(_bf)
        ruT = np.ascontiguousarray(np.asarray(routed_up[c], np.float32).T).astype(_bf)
        rdT = np.ascontiguousarray(np.asarray(routed_down[c], np.float32).T).astype(_bf)
        wch = np.concatenate([_pack_chunks(rgT, ruT, rdT), shared_packs[s]])
        in_maps.append({"xr": xr, "xs": xs_, "wch": wch})

    # ---- build + run on 8 cores -----------------------------------------
    if cr not in _NC_CACHE:
        _NC_CACHE[cr] = build_nc(cr)
    nc = _NC_CACHE[cr]

    trace = bool(int(os.environ.get("MOE_TRACE", "0")))
    kw = {}
    if trace:
        _install_ntff_hook()
        kw = dict(trace=True, trace_cores=list(range(N_CORES)))
    res = run_bass_kernel_spmd(nc, in_maps, core_ids=list(range(N_CORES)), **kw)
    if trace:
        print(f"HW exec time: {res.exec_time_ns} ns")

    # ---- host combine ----------------------------------------------------
    out = np.zeros((T, H), np.float32)
    for c in range(N_CORES):
        toks = tok_lists[c]
        yrT = res.results[c]["yr"].astype(np.float32)                # [H, cr]
        out[toks] += yrT[:, :len(toks)].T * tok_weights[c][:, None]
        q = c % (N_CORES // NS)
        out[q * CS:(q + 1) * CS] += res.results[c]["ys"].astype(np.float32).T / NS
    return out.reshape(B, S, H)


# revision 16
# speedup vs baseline: 1.1046x; 1.0299x over previous
"""DeepSeek-MoE layer on 8 TRN2 NeuronCores.

Strategy (expert-parallel, host-side dispatch):
  - Router (x @ gate_w.T, softmax, top-2) computed on host — it *is* the
    sharding decision (~0.02% of total FLOPs).
  - Core c computes routed expert c's SwiGLU FFN over the tokens routed to
    it (gathered+padded to a fixed capacity), plus a 512-token chunk of
    shared expert c//4 (each shared expert covers all 2048 tokens, split
    over 4 cores).
  - All matmuls in bf16 (fp32 PSUM accumulation). Combine weights /
    scatter-add applied on host in fp32.

Device kernel layout:
  - Tokens live on the matmul free axis (x stored transposed [H, C]).
  - Weights are streamed as 768KB "4-mi chunks" pre-packed on the host
    into their exact SBUF image, ordered gate/up/down interleaved in
    consumption order — the PE is weight-DMA-paced from ~5us on with no
    phase barriers.
  - Fully fused inner loop per 128-row slice mi of I: 6 gate MMs, 6 up
    MMs, silu (ACT), mul->bf16 (DVE), 6 down MMs accumulating into 6
    PSUM banks (one per 128-row slice of H).
  - Routed capacity is split into two equal column tiles (per-MM cost is
    max(N/2.4GHz, ~64ns floor), so twin tiles beat 512+tail).
  - Output stores go on the ACT HWDGE ring so they never head-of-line
    block weight loads on the sync ring.
"""
import os
import sys
import types

import numpy as np
import ml_dtypes

import concourse.bass as bass
import concourse.tile as tile
import concourse.mybir as mybir
from concourse import bacc
from concourse.bass_utils import run_bass_kernel_spmd

# ---- problem constants (DeepSeekMoE: B=2,S=1024,H=768,I=3072,E=8,NS=2,k=2) --
H = 768          # hidden
I = 3072         # intermediate
E = 8            # routed experts
NS = 2           # shared experts
TOP_K = 2
N_CORES = 8
KH = H // 128    # 6 k-tiles over H
KI = I // 128    # 24 mi-tiles over I
NCH = KI // 4    # 6 weight chunk-groups (4 mi each)
CS = 2048 * NS // N_CORES  # shared-expert tokens per core = 512

BF16 = mybir.dt.bfloat16
F32 = mybir.dt.float32
_bf = ml_dtypes.bfloat16


def _install_ntff_hook():
    """Provide antenv.axon_hooks (missing on this image) so trace=True works."""
    if "antenv.axon_hooks" in sys.modules:
        return
    try:
        from trn_agent_boot.trn_boot import _ntff_profile_via_ctypes
        hook = _ntff_profile_via_ctypes("/opt/axon/libaxon_pjrt.so")
    except Exception:
        hook = None
    mod = types.ModuleType("antenv.axon_hooks")
    mod.get_axon_ntff_profile_hook = lambda: hook
    sys.modules["antenv.axon_hooks"] = mod


def _col_tiles(c):
    if c <= 512:
        return [(0, c)]
    half = (c // 2 + 31) // 32 * 32
    return [(0, half), (half, c - half)]


def _ffn_job(nc, wpool, hpool, sgpool, gupool, ypool, ystage,
             x_sb, wchunks, base, y_dram, n_tiles):
    """One SwiGLU FFN: y = (silu(x Wg) * (x Wu)) Wd for one expert.

    wchunks[base + 3c + {0,1,2}] are the gate/up/down weight chunks for
    mi-group c, pre-packed on host as the exact [128, 3072] SBUF image.
    """
    gu_t = {}
    wd_t = {}
    for c in range(NCH):
        tg = wpool.tile([128, KH, 4, 128], BF16, tag="w")
        nc.sync.dma_start(out=tg, in_=wchunks[base + 3 * c + 0, :, :]
                          .rearrange("p (k j m) -> p k j m", k=KH, j=4))
        tu = wpool.tile([128, KH, 4, 128], BF16, tag="w")
        nc.sync.dma_start(out=tu, in_=wchunks[base + 3 * c + 1, :, :]
                          .rearrange("p (k j m) -> p k j m", k=KH, j=4))
        gu_t[c] = (tg, tu)
    for c in range(NCH):
        td = wpool.tile([128, 4, H], BF16, tag="w")
        nc.sync.dma_start(out=td, in_=wchunks[base + 3 * c + 2, :, :]
                          .rearrange("p (j i) -> p j i", j=4))
        wd_t[c] = td

    for ti, (n0, nsz) in enumerate(n_tiles):
        # gate/up + silu*mul for all 24 mi (4 PSUM banks -> the silu/mul
        # round-trip never stalls the next mi's matmuls)
        hs = {}
        for c in range(NCH):
            tg, tu = gu_t[c]
            for j in range(4):
                g = gupool.tile([128, 512], F32, tag="gu")
                u = gupool.tile([128, 512], F32, tag="gu")
                for k in range(KH):
                    nc.tensor.matmul(
                        g[:, :nsz], tg[:, k, j, :], x_sb[:, k, n0:n0 + nsz],
                        start=(k == 0), stop=(k == KH - 1))
                for k in range(KH):
                    nc.tensor.matmul(
                        u[:, :nsz], tu[:, k, j, :], x_sb[:, k, n0:n0 + nsz],
                        start=(k == 0), stop=(k == KH - 1))
                sg = sgpool.tile([128, 512], F32, tag="sg")
                nc.scalar.activation(sg[:, :nsz], g[:, :nsz],
                                     mybir.ActivationFunctionType.Silu)
                h = hpool.tile([128, 512], BF16, tag="h")
                nc.vector.tensor_mul(h[:, :nsz], sg[:, :nsz], u[:, :nsz])
                hs[4 * c + j] = h
        # down proj in two passes of 3 output banks (3+4 <= 8 PSUM banks)
        for half in range(2):
            ys = [ypool.tile([128, 512], F32, tag="y", name=f"y{hj}")
                  for hj in range(3)]
            for mi in range(KI):
                for t in range(3):
                    hj = 3 * half + t
                    nc.tensor.matmul(
                        ys[t][:, :nsz],
                        wd_t[mi // 4][:, mi % 4, hj * 128:(hj + 1) * 128],
                        hs[mi][:, :nsz],
                        start=(mi == 0), stop=(mi == KI - 1))
            # copies alternate ACT/DVE, stores alternate ACT/sync HWDGE
            # rings, so the drain-out never serializes on one engine
            for t in range(3):
                hj = 3 * half + t
                yst = ystage.tile([128, 512], BF16, tag="yst")
                if t % 2 == 0:
                    nc.scalar.copy(yst[:, :nsz], ys[t][:, :nsz])
                else:
                    nc.vector.tensor_copy(yst[:, :nsz], ys[t][:, :nsz])
                eng = nc.scalar if t % 2 == 0 else nc.sync
                eng.dma_start(
                    out=y_dram[hj * 128:(hj + 1) * 128, n0:n0 + nsz],
                    in_=yst[:, :nsz])


def build_nc(cr):
    """Build the SPMD program. cr = routed-token capacity (multiple of 32)."""
    nc = bacc.Bacc(None, target_bir_lowering=False)
    xr = nc.dram_tensor("xr", [H, cr], BF16, kind="ExternalInput")
    xs = nc.dram_tensor("xs", [H, CS], BF16, kind="ExternalInput")
    wch = nc.dram_tensor("wch", [6 * NCH, 128, KH * 512], BF16,
                         kind="ExternalInput")
    yr = nc.dram_tensor("yr", [H, cr], BF16, kind="ExternalOutput")
    ys = nc.dram_tensor("ys", [H, CS], BF16, kind="ExternalOutput")

    with tile.TileContext(nc) as tc:
        with tc.tile_pool(name="wpool", bufs=23) as wpool, \
             tc.tile_pool(name="xpool", bufs=1) as xpool, \
             tc.tile_pool(name="hpool", bufs=26) as hpool, \
             tc.tile_pool(name="sgpool", bufs=4) as sgpool, \
             tc.tile_pool(name="ystage", bufs=4) as ystage, \
             tc.tile_pool(name="gupool", bufs=4, space="PSUM") as gupool, \
             tc.tile_pool(name="ypool", bufs=3, space="PSUM") as ypool:
            # x loads ride the ACT HWDGE ring so they stream in parallel
            # with the weight chunks on the sync ring at kernel start
            xr_sb = xpool.tile([128, KH, cr], BF16, tag="xr")
            xs_sb = xpool.tile([128, KH, CS], BF16, tag="xs")
            for k in range(KH):
                nc.scalar.dma_start(
                    out=xr_sb[:, k, :], in_=xr[k * 128:(k + 1) * 128, :])
            for k in range(KH):
                nc.scalar.dma_start(
                    out=xs_sb[:, k, :], in_=xs[k * 128:(k + 1) * 128, :])

            _ffn_job(nc, wpool, hpool, sgpool, gupool, ypool, ystage,
                     xr_sb, wch, 0, yr, _col_tiles(cr))
            _ffn_job(nc, wpool, hpool, sgpool, gupool, ypool, ystage,
                     xs_sb, wch, 3 * NCH, ys, _col_tiles(CS))
    nc.finalize()
    return nc


def _chunk_gu(wT):
    """[H, I] lhsT-layout weight -> [NCH, 128, 3072] SBUF chunk images.
    chunk[c][p, k*512 + j*128 + m] = wT[k*128 + p, (4c+j)*128 + m]"""
    a = wT.reshape(KH, 128, NCH, 4 * 128)        # [k, p, c, jm]
    return np.ascontiguousarray(a.transpose(2, 1, 0, 3)).reshape(NCH, 128, KH * 512)


def _chunk_wd(wdT):
    """[I, H] lhsT-layout down weight -> [NCH, 128, 3072] chunk images.
    chunk[c][p, j*768 + i] = wdT[(4c+j)*128 + p, i]"""
    a = wdT.reshape(NCH, 4, 128, H)              # [c, j, p, i]
    return np.ascontiguousarray(a.transpose(0, 2, 1, 3)).reshape(NCH, 128, 4 * H)


def _pack_chunks(gT, uT, dT):
    """Interleave gate/up/down chunks in consumption order -> [18, 128, 3072]."""
    g = _chunk_gu(gT)
    u = _chunk_gu(uT)
    d = _chunk_wd(dT)
    out = np.empty((3 * NCH, 128, KH * 512), _bf)
    out[0::3] = g
    out[1::3] = u
    out[2::3] = d
    return out


_NC_CACHE = {}


def kernel(hidden_states, gate_w, shared_gate, shared_up, shared_down,
           routed_gate, routed_up, routed_down):
    B, S, _ = hidden_states.shape
    T = B * S
    x = np.asarray(hidden_states, np.float32).reshape(T, H)

    # ---- host router (mirrors reference math; fp64 softmax for stability) --
    logits = x @ np.asarray(gate_w, np.float32).T                    # [T, E]
    lg = logits.astype(np.float64)
    sc = np.exp(lg - lg.max(1, keepdims=True))
    sc /= sc.sum(1, keepdims=True)
    topk_idx = np.argsort(-sc, axis=1, kind="stable")[:, :TOP_K]     # [T, k]
    topk_w = np.take_along_axis(sc, topk_idx, axis=1)
    topk_w = topk_w / (topk_w.sum(1, keepdims=True) + 1e-8)          # [T, k]

    tok_lists = []
    tok_weights = []
    for e in range(E):
        sel = (topk_idx == e)
        toks = np.where(sel.any(1))[0]
        w = (topk_w * sel)[toks].sum(1).astype(np.float32)
        tok_lists.append(toks)
        tok_weights.append(w)
    max_n = max(len(t) for t in tok_lists)
    cr = max(64, -(-max_n // 32) * 32)

    # ---- per-core inputs -------------------------------------------------
    x_bf = x.astype(_bf)
    shared_packs = []
    for s in range(NS):
        sgT = np.ascontiguousarray(np.asarray(shared_gate[s], np.float32).T).astype(_bf)
        suT = np.ascontiguousarray(np.asarray(shared_up[s], np.float32).T).astype(_bf)
        sdT = np.ascontiguousarray(np.asarray(shared_down[s], np.float32).T).astype(_bf)
        shared_packs.append(_pack_chunks(sgT, suT, sdT))

    in_maps = []
    for c in range(N_CORES):
        toks = tok_lists[c]
        xr = np.zeros((H, cr), _bf)
        xr[:, :len(toks)] = x_bf[toks].T
        s = c // (N_CORES // NS)
        q = c % (N_CORES // NS)
        xs_ = np.ascontiguousarray(x_bf[q * CS:(q + 1) * CS].T)
        rgT = np.ascontiguousarray(np.asarray(routed_gate[c], np.float32).T).astype(_bf)
        ruT = np.ascontiguousarray(np.asarray(routed_up[c], np.float32).T).astype(_bf)
        rdT = np.ascontiguousarray(np.asarray(routed_down[c], np.float32).T).astype(_bf)
        wch = np.concatenate([_pack_chunks(rgT, ruT, rdT), shared_packs[s]])
        in_maps.append({"xr": xr, "xs": xs_, "wch": wch})

    # ---- build + run on 8 cores -----------------------------------------
    if cr not in _NC_CACHE:
        _NC_CACHE[cr] = build_nc(cr)
    nc = _NC_CACHE[cr]

    trace = bool(int(os.environ.get("MOE_TRACE", "0")))
    kw = {}
    if trace:
        _install_ntff_hook()
        kw = dict(trace=True, trace_cores=list(range(N_CORES)))
    res = run_bass_kernel_spmd(nc, in_maps, core_ids=list(range(N_CORES)), **kw)
    if trace:
        print(f"HW exec time: {res.exec_time_ns} ns")

    # ---- host combine ----------------------------------------------------
    out = np.zeros((T, H), np.float32)
    for c in range(N_CORES):
        toks = tok_lists[c]
        yrT = res.results[c]["yr"].astype(np.float32)                # [H, cr]
        out[toks] += yrT[:, :len(toks)].T * tok_weights[c][:, None]
        q = c % (N_CORES // NS)
        out[q * CS:(q + 1) * CS] += res.results[c]["ys"].astype(np.float32).T / NS
    return out.reshape(B, S, H)
